# revision 2
# baseline (speedup 1.0000x reference)
"""DWA LanguageModel layer on 8 trn2 NeuronCores.

Strategy:
  - Tokens (B=1024) data-parallel across 8 cores (128 each).
  - Keys (pool @ W_K, token-independent) N-sharded: each core computes
    normalized+aspect-weighted keys for its 128 pool rows, AllGather.
  - Low-rank update never materializes UV: h_delta[b] = sum_n alpha[b,n]
    U_n (V_n z_b) via dense matmuls over nr=(n,r) with alpha sparsity
    handled by top-16 masking (masked alpha entries are exactly 0).
  - Top-16 threshold per token via vector.max + match_replace + vector.max.
  - alpha normalization deferred to the final combine (linear in alpha).
"""
import sys

sys.path.insert(0, "/opt/trn_rl_repo")
import numpy as np
import ml_dtypes

import concourse.bass as bass
import concourse.mybir as mybir
import concourse.tile as tile
from concourse import bacc
from concourse.bass_utils import run_bass_kernel_spmd
from concourse.masks import make_identity

F32 = mybir.dt.float32
BF16 = mybir.dt.bfloat16
AF = mybir.ActivationFunctionType
ALU = mybir.AluOpType

NCORES = 8
B = 1024            # tokens
BL = B // NCORES    # tokens per core = 128
D_MODEL = 512
N = 1024            # pool rows
D = 16384           # pool cols
S = 2
DK = 64
SDK = S * DK        # 128
R = 8
NR = N * R          # 8192
K_MAX = 16
LAMBDA_SHARP = 5.0
LN_EPS = 1e-5
U_END = D_MODEL * R          # 4096
V_END = U_END + R * D_MODEL  # 8192
B_END = V_END + D_MODEL      # 8704
NL = N // NCORES             # pool rows per core = 128

# dtype config: keys path and dynamic (post-alpha) path
DT_KEYS = F32
DT_DYN = BF16

LAST_EXEC_NS = None
TRACE = True
NO_CC = False
LEVEL = 9  # bisect: 0=io 1=keys 2=+AG 3=+scores 4=alpha 5=s 6=no-bias 9=full


def _np_dt(dt):
    return {F32: np.float32, BF16: ml_dtypes.bfloat16}[dt]


def _build(tau_f, w0_f, w1_f, gamma_f):
    nc = bacc.Bacc("TRN2", target_bir_lowering=False, debug=False,
                   num_devices=NCORES)

    # ---- I/O ----
    z_d = nc.dram_tensor("z", [BL, D_MODEL], F32, kind="ExternalInput")
    zt_d = nc.dram_tensor("zt", [D_MODEL, BL], F32, kind="ExternalInput")
    poolT_d = nc.dram_tensor("poolT", [D, NL], DT_KEYS, kind="ExternalInput")
    wk_d = nc.dram_tensor("wk", [D, SDK], DT_KEYS, kind="ExternalInput")
    wq_d = nc.dram_tensor("wq", [D_MODEL, SDK], F32, kind="ExternalInput")
    vt_d = nc.dram_tensor("vt", [D_MODEL, NR], DT_DYN, kind="ExternalInput")
    up_d = nc.dram_tensor("up", [NR, D_MODEL], DT_DYN, kind="ExternalInput")
    pb_d = nc.dram_tensor("pb", [N, D_MODEL], DT_DYN, kind="ExternalInput")
    wbt_d = nc.dram_tensor("wbt", [D_MODEL, D_MODEL], F32, kind="ExternalInput")
    gb_d = nc.dram_tensor("gb", [BL, D_MODEL], F32, kind="ExternalInput")
    ls_d = nc.dram_tensor("ls", [BL, D_MODEL], F32, kind="ExternalInput")
    lb_d = nc.dram_tensor("lb", [BL, D_MODEL], F32, kind="ExternalInput")
    out_d = nc.dram_tensor("out", [BL, D_MODEL], F32, kind="ExternalOutput")

    with tile.TileContext(nc) as tc:
        with (
            tc.tile_pool(name="sb", bufs=1) as sb,            # persistent tiles
            tc.tile_pool(name="sbs", bufs=4) as sbs,          # streamed tiles
            tc.tile_pool(name="dram", bufs=1, space="DRAM") as dram,
        ):
            _emit(nc, tc, sb, sbs, dram, locals(), tau_f, w0_f, w1_f, gamma_f,
                  z_d, zt_d, poolT_d, wk_d, wq_d, vt_d, up_d, pb_d, wbt_d,
                  gb_d, ls_d, lb_d, out_d)

    nc.compile()
    return nc


def _emit(nc, tc, sb, sbs, dram, _l, tau_f, w0_f, w1_f, gamma_f,
          z_d, zt_d, poolT_d, wk_d, wq_d, vt_d, up_d, pb_d, wbt_d,
          gb_d, ls_d, lb_d, out_d):
    if LEVEL <= 0:
        z0 = sb.tile([BL, D_MODEL], F32, tag="z0")
        nc.sync.dma_start(z0[:], z_d[:])
        pt0 = sb.tile([128, NL], DT_KEYS, tag="pt0")
        nc.sync.dma_start(pt0[:], poolT_d[0:128, :])
        vt0 = sb.tile([128, 512], DT_DYN, tag="vt0")
        nc.sync.dma_start(vt0[:], vt_d[0:128, 0:512])
        up0 = sb.tile([128, 512], DT_DYN, tag="up0")
        nc.sync.dma_start(up0[:], up_d[0:128, :])
        o0 = sb.tile([BL, D_MODEL], F32, tag="o0")
        nc.vector.tensor_scalar_mul(o0[:], z0[:], 2.0)
        nc.sync.dma_start(out_d[:], o0[:])
        return

    # ---------- small persistent loads ----------
    ident = sb.tile([128, 128], F32, tag="ident")
    make_identity(nc, ident[:])
    if DT_DYN != F32:
        identd = sb.tile([128, 128], DT_DYN, tag="identd")
        make_identity(nc, identd[:])
    else:
        identd = ident
    z_sb = sb.tile([BL, D_MODEL], F32, tag="z")
    nc.sync.dma_start(z_sb[:], z_d[:])
    gb_sb = sb.tile([BL, D_MODEL], F32, tag="gb")
    nc.sync.dma_start(gb_sb[:], gb_d[:])
    ls_sb = sb.tile([BL, D_MODEL], F32, tag="ls")
    nc.sync.dma_start(ls_sb[:], ls_d[:])
    lb_sb = sb.tile([BL, D_MODEL], F32, tag="lb")
    nc.sync.dma_start(lb_sb[:], lb_d[:])
    # zb = z + gamma * b_base   (gb = gamma*b_base replicated)
    zb_sb = sb.tile([BL, D_MODEL], F32, tag="zb")
    nc.vector.tensor_add(zb_sb[:], z_sb[:], gb_sb[:])

    # zt: [128, 4*128], chunk c holds rows a=c*128..(c+1)*128 of z^T
    zt_sb = sb.tile([128, D_MODEL], F32, tag="zt")
    for c in range(4):
        nc.sync.dma_start(zt_sb[:, c * 128:(c + 1) * 128],
                          zt_d[c * 128:(c + 1) * 128, :])
    wq_sb = sb.tile([128, 4 * SDK], F32, tag="wq")
    for c in range(4):
        nc.sync.dma_start(wq_sb[:, c * SDK:(c + 1) * SDK],
                          wq_d[c * 128:(c + 1) * 128, :])
    if DT_DYN != F32:
        ztd_sb = sb.tile([128, D_MODEL], DT_DYN, tag="ztd")
        nc.vector.tensor_copy(ztd_sb[:], zt_sb[:])
    else:
        ztd_sb = zt_sb

    # ---------- phase A: keys (N-sharded) ----------
    with tc.tile_pool(name="psA", bufs=1, space="PSUM") as psA:
        keys_ps = psA.tile([NL, SDK], F32, tag="keys")
        KB = 16  # D-chunks per DMA batch
        nblk = D // (128 * KB)
        for blk in range(nblk):
            lo, hi = blk * 128 * KB, (blk + 1) * 128 * KB
            pt = sbs.tile([128, KB * NL], DT_KEYS, tag="pt")
            nc.sync.dma_start(
                pt[:].rearrange("p (i n) -> p i n", i=KB),
                poolT_d[lo:hi, :].rearrange("(i p) n -> p i n", p=128))
            wkt = sbs.tile([128, KB * SDK], DT_KEYS, tag="wkt")
            nc.sync.dma_start(
                wkt[:].rearrange("p (i n) -> p i n", i=KB),
                wk_d[lo:hi, :].rearrange("(i p) n -> p i n", p=128))
            for i in range(KB):
                d = blk * KB + i
                nc.tensor.matmul(keys_ps[:],
                                 pt[:, i * NL:(i + 1) * NL],
                                 wkt[:, i * SDK:(i + 1) * SDK],
                                 start=(d == 0), stop=(d == D // 128 - 1))
        # normalize along free (two 64-wide aspect groups), fold w_s
        ksq = sb.tile([NL, S], F32, tag="ksq")
        ksc = sb.tile([NL, SDK], F32, tag="ksc")  # scratch square out
        for s in range(S):
            nc.scalar.activation(ksc[:, s * DK:(s + 1) * DK],
                                 keys_ps[:, s * DK:(s + 1) * DK],
                                 AF.Square,
                                 accum_out=ksq[:, s:s + 1])
        knorm = sb.tile([NL, S], F32, tag="knorm")
        nc.scalar.activation(knorm[:], ksq[:], AF.Sqrt)
        nc.vector.tensor_scalar_add(knorm[:], knorm[:], 1e-8)
        krec = sb.tile([NL, S], F32, tag="krec")
        nc.vector.reciprocal(krec[:], knorm[:])
        kn_w = sb.tile([NL, SDK], F32, tag="kn_w")
        for s, w_s in ((0, w0_f), (1, w1_f)):
            nc.vector.tensor_scalar(
                kn_w[:, s * DK:(s + 1) * DK],
                keys_ps[:, s * DK:(s + 1) * DK],
                krec[:, s:s + 1], float(w_s),
                op0=ALU.mult, op1=ALU.mult)

        if LEVEL <= 1:
            nc.sync.dma_start(out_d[:, :SDK], kn_w[:])
            return

        # ---------- phase B: queries ----------
        q_ps = psA.tile([BL, SDK], F32, tag="q")
        for c in range(4):
            nc.tensor.matmul(q_ps[:],
                             zt_sb[:, c * 128:(c + 1) * 128],
                             wq_sb[:, c * SDK:(c + 1) * SDK],
                             start=(c == 0), stop=(c == 3))
        qsq = sb.tile([BL, S], F32, tag="qsq")
        qsc = sb.tile([BL, SDK], F32, tag="qsc")
        for s in range(S):
            nc.scalar.activation(qsc[:, s * DK:(s + 1) * DK],
                                 q_ps[:, s * DK:(s + 1) * DK],
                                 AF.Square,
                                 accum_out=qsq[:, s:s + 1])
        qnorm = sb.tile([BL, S], F32, tag="qnorm")
        nc.scalar.activation(qnorm[:], qsq[:], AF.Sqrt)
        nc.vector.tensor_scalar_add(qnorm[:], qnorm[:], 1e-8)
        qrec = sb.tile([BL, S], F32, tag="qrec")
        nc.vector.reciprocal(qrec[:], qnorm[:])
        q_n = sb.tile([BL, SDK], F32, tag="q_n")
        for s in range(S):
            nc.vector.tensor_scalar(
                q_n[:, s * DK:(s + 1) * DK],
                q_ps[:, s * DK:(s + 1) * DK],
                qrec[:, s:s + 1], None, op0=ALU.mult)

    # ---------- AllGather keys ----------
    cc_in = dram.tile([NL, SDK], F32)
    cc_out = dram.tile([N, SDK], F32)
    nc.sync.dma_start(cc_in[:], kn_w[:])
    if NO_CC:
        for c in range(NCORES):
            nc.sync.dma_start(cc_out[c * NL:(c + 1) * NL, :], cc_in[:])
    else:
        nc.gpsimd.collective_compute(
            "AllGather", ALU.bypass,
            replica_groups=[list(range(NCORES))],
            ins=[cc_in[:].opt()], outs=[cc_out[:].opt()],
        )

    if LEVEL <= 2:
        gat = sb.tile([NL, SDK], F32, tag="gat")
        nc.sync.dma_start(gat[:], cc_out[0:NL, :])
        nc.sync.dma_start(out_d[:, :SDK], gat[:])
        return

    with tc.tile_pool(name="psC", bufs=2, space="PSUM") as psC:
        # transpose q_n -> qnT [sdk, b]
        qn_tp = psC.tile([SDK, BL], F32, tag="ktr")
        nc.tensor.transpose(qn_tp[:], q_n[:], ident[:])
        qnT = sb.tile([SDK, BL], F32, tag="qnT")
        nc.scalar.activation(qnT[:], qn_tp[:], AF.Copy)

        # load gathered keys, transpose to knT [sdk, n]
        knT = sb.tile([SDK, N], F32, tag="knT")
        for c in range(NCORES):
            kc = sbs.tile([NL, SDK], F32, tag="kc")
            nc.sync.dma_start(kc[:], cc_out[c * NL:(c + 1) * NL, :])
            ktp = psC.tile([SDK, NL], F32, tag="ktr")
            nc.tensor.transpose(ktp[:], kc[:], ident[:])
            nc.scalar.activation(knT[:, c * NL:(c + 1) * NL],
                                 ktp[:], AF.Copy)

        # ---------- scores [b, n] ----------
        # sum over both aspects = single K=128 contraction (w folded in keys)
        scores = sb.tile([BL, N], F32, tag="scores")
        for j in range(2):
            sc_ps = psC.tile([BL, 512], F32, tag="sc")
            nc.tensor.matmul(sc_ps[:], qnT[:],
                             knT[:, j * 512:(j + 1) * 512],
                             start=True, stop=True)
            nc.scalar.activation(scores[:, j * 512:(j + 1) * 512],
                                 sc_ps[:], AF.Copy)

    if LEVEL <= 3:
        nc.sync.dma_start(out_d[:], scores[:, :D_MODEL])
        return

    # ---------- top-16 threshold + alpha ----------
    m8a = sb.tile([BL, 8], F32, tag="m8a")
    nc.vector.max(out=m8a[:], in_=scores[:])
    s_mr = sb.tile([BL, N], F32, tag="s_mr")
    nc.vector.match_replace(out=s_mr[:], in_to_replace=m8a[:],
                            in_values=scores[:], imm_value=-1e30)
    m8b = sb.tile([BL, 8], F32, tag="m8b")
    nc.vector.max(out=m8b[:], in_=s_mr[:])
    # threshold = 16th largest = m8b[:, 7]
    sig_b = sb.tile([BL, 1], F32, tag="sig_b")
    nc.vector.memset(sig_b[:], float(-LAMBDA_SHARP * tau_f))
    sig = sb.tile([BL, N], F32, tag="sig")
    nc.scalar.activation(sig[:], scores[:], AF.Sigmoid,
                         scale=LAMBDA_SHARP, bias=sig_b[:])
    ex = sb.tile([BL, N], F32, tag="ex")
    nc.scalar.activation(ex[:], scores[:], AF.Exp)
    ge = sb.tile([BL, N], F32, tag="ge")
    nc.vector.tensor_mul(ge[:], sig[:], ex[:])
    alpha = sb.tile([BL, N], F32, tag="alpha")
    den = sb.tile([BL, 1], F32, tag="den")
    nc.vector.scalar_tensor_tensor(
        out=alpha[:], in0=scores[:], scalar=m8b[:, 7:8], in1=ge[:],
        op0=ALU.is_ge, op1=ALU.mult, accum_out=den[:])
    nc.vector.tensor_scalar_add(den[:], den[:], 1e-8)
    rden = sb.tile([BL, 1], F32, tag="rden")
    nc.vector.reciprocal(rden[:], den[:])

    if LEVEL <= 4:
        nc.sync.dma_start(out_d[:], alpha[:, :D_MODEL])
        return

    s_sb = sb.tile([BL, NR], DT_DYN, tag="s_sb")

    with tc.tile_pool(name="psE", bufs=1, space="PSUM") as psE:
        # ---------- t = z @ V^T, s = alpha * t ----------
        for j in range(16):
            t_ps = psE.tile([BL, 512], F32, tag="t")
            vtt = sbs.tile([128, 4 * 512], DT_DYN, tag="vtt")
            nc.sync.dma_start(
                vtt[:].rearrange("p (c f) -> p c f", c=4),
                vt_d[:, j * 512:(j + 1) * 512]
                .rearrange("(c p) f -> p c f", p=128))
            for c in range(4):
                nc.tensor.matmul(t_ps[:],
                                 ztd_sb[:, c * 128:(c + 1) * 128],
                                 vtt[:, c * 512:(c + 1) * 512],
                                 start=(c == 0), stop=(c == 3))
            nc.vector.tensor_tensor(
                out=s_sb[:, j * 512:(j + 1) * 512]
                    .rearrange("p (n r) -> p n r", r=R),
                in0=t_ps[:].rearrange("p (n r) -> p n r", r=R),
                in1=alpha[:, j * 64:(j + 1) * 64]
                    .unsqueeze(2).broadcast_to([BL, 64, R]),
                op=ALU.mult)

        if LEVEL <= 5:
            nc.gpsimd.dma_start(out_d[:], s_sb[:, :D_MODEL])
            return

        # ---------- h1 = sT @ U (+ alphaT @ bias), unnormalized ----
        h1_ps = psE.tile([BL, D_MODEL], F32, tag="h1")
        for gb in range(16):
            upt = sbs.tile([128, 4 * D_MODEL], DT_DYN, tag="upt")
            nc.sync.dma_start(
                upt[:].rearrange("p (i c) -> p i c", i=4),
                up_d[gb * 512:(gb + 1) * 512, :]
                .rearrange("(i p) c -> p i c", p=128))
            for i in range(4):
                g = gb * 4 + i
                st_tp = psE.tile([128, 128], DT_DYN, tag="tr")
                nc.tensor.transpose(st_tp[:],
                                    s_sb[:, g * 128:(g + 1) * 128],
                                    identd[:])
                sT = sbs.tile([128, 128], DT_DYN, tag="sT")
                nc.scalar.activation(sT[:], st_tp[:], AF.Copy)
                nc.tensor.matmul(h1_ps[:], sT[:],
                                 upt[:, i * D_MODEL:(i + 1) * D_MODEL],
                                 start=(g == 0), stop=(LEVEL <= 6 and g == 63))
        if LEVEL >= 7:
            for c in range(NCORES):
                al_tp = psE.tile([128, 128], F32, tag="atr")
                nc.tensor.transpose(al_tp[:],
                                    alpha[:, c * 128:(c + 1) * 128],
                                    ident[:])
                aT = sbs.tile([128, 128], DT_DYN, tag="aT")
                nc.scalar.activation(aT[:], al_tp[:], AF.Copy)
                pbt = sbs.tile([128, D_MODEL], DT_DYN, tag="pbt")
                nc.sync.dma_start(pbt[:], pb_d[c * NL:(c + 1) * NL, :])
                nc.tensor.matmul(h1_ps[:], aT[:], pbt[:],
                                 start=False, stop=(c == NCORES - 1))

        # ---------- h2 = z @ W_base^T ----------
        h2_ps = psE.tile([BL, D_MODEL], F32, tag="h2")
        for c in range(4):
            wbt = sbs.tile([128, D_MODEL], F32, tag="wbt")
            nc.sync.dma_start(wbt[:], wbt_d[c * 128:(c + 1) * 128, :])
            nc.tensor.matmul(h2_ps[:],
                             zt_sb[:, c * 128:(c + 1) * 128],
                             wbt[:], start=(c == 0), stop=(c == 3))

        # ---------- combine ----------
        A_sb = sb.tile([BL, D_MODEL], F32, tag="A")
        nc.vector.tensor_scalar(A_sb[:], h1_ps[:], rden[:], None,
                                op0=ALU.mult)
        nc.vector.tensor_add(A_sb[:], A_sb[:], h2_ps[:])

    # ---------- layernorm ----------
    x_sb = sb.tile([BL, D_MODEL], F32, tag="x")
    nc.vector.scalar_tensor_tensor(
        out=x_sb[:], in0=A_sb[:], scalar=float(gamma_f), in1=zb_sb[:],
        op0=ALU.mult, op1=ALU.add)
    mean = sb.tile([BL, 1], F32, tag="mean")
    nc.vector.reduce_sum(mean[:], x_sb[:], axis=mybir.AxisListType.X)
    nc.vector.tensor_scalar_mul(mean[:], mean[:], 1.0 / D_MODEL)
    xc = sb.tile([BL, D_MODEL], F32, tag="xc")
    nc.vector.tensor_scalar(xc[:], x_sb[:], mean[:], None,
                            op0=ALU.subtract)
    xsq = sb.tile([BL, D_MODEL], F32, tag="xsq")
    ssq = sb.tile([BL, 1], F32, tag="ssq")
    nc.scalar.activation(xsq[:], xc[:], AF.Square, accum_out=ssq[:])
    vare = sb.tile([BL, 1], F32, tag="vare")
    nc.vector.tensor_scalar(vare[:], ssq[:], 1.0 / D_MODEL, LN_EPS,
                            op0=ALU.mult, op1=ALU.add)
    sd = sb.tile([BL, 1], F32, tag="sd")
    nc.scalar.activation(sd[:], vare[:], AF.Sqrt)
    rstd = sb.tile([BL, 1], F32, tag="rstd")
    nc.vector.reciprocal(rstd[:], sd[:])
    y1 = sb.tile([BL, D_MODEL], F32, tag="y1")
    nc.vector.tensor_scalar(y1[:], xc[:], rstd[:], None, op0=ALU.mult)
    out_sb = sb.tile([BL, D_MODEL], F32, tag="out_sb")
    nc.vector.tensor_mul(out_sb[:], y1[:], ls_sb[:])
    nc.vector.tensor_add(out_sb[:], out_sb[:], lb_sb[:])
    nc.sync.dma_start(out_d[:], out_sb[:])


def kernel(z, pool_vectors, W_Q, W_K, aspect_logits, tau,
           W_base, b_base, gamma, ln_scale, ln_bias):
    global LAST_EXEC_NS
    z = np.asarray(z, np.float32)
    pool = np.asarray(pool_vectors, np.float32)
    W_Q = np.asarray(W_Q, np.float32)
    W_K = np.asarray(W_K, np.float32)
    aspect_logits = np.asarray(aspect_logits, np.float32)
    tau_f = float(np.asarray(tau))
    W_base = np.asarray(W_base, np.float32)
    b_base = np.asarray(b_base, np.float32)
    gamma_f = float(np.asarray(gamma))
    ln_scale = np.asarray(ln_scale, np.float32)
    ln_bias = np.asarray(ln_bias, np.float32)

    e = np.exp(aspect_logits - aspect_logits.max())
    w = e / e.sum()
    w0_f, w1_f = float(w[0]), float(w[1])

    nc = _build(tau_f, w0_f, w1_f, gamma_f)

    # ---- host-side layout prep ----
    np_keys = _np_dt(DT_KEYS)
    np_dyn = _np_dt(DT_DYN)
    wk_cat = np.concatenate([W_K[0], W_K[1]], axis=1).astype(np_keys)  # [D, 128]
    wq_cat = np.concatenate([W_Q[0], W_Q[1]], axis=1).astype(np.float32)
    # V^T: [e, n*R + r] = pool[n, 4096 + r*512 + e]
    vt = np.ascontiguousarray(
        pool[:, U_END:V_END].reshape(N, R, D_MODEL).transpose(2, 0, 1)
        .reshape(D_MODEL, NR)).astype(np_dyn)
    # U_perm: [n*R + r, c] = pool[n, c*R + r]
    up = np.ascontiguousarray(
        pool[:, :U_END].reshape(N, D_MODEL, R).transpose(0, 2, 1)
        .reshape(NR, D_MODEL)).astype(np_dyn)
    pb = np.ascontiguousarray(pool[:, V_END:B_END]).astype(np_dyn)  # [N, 512]
    wbt = np.ascontiguousarray(W_base.T)  # [a, c]
    gb = np.broadcast_to((gamma_f * b_base).astype(np.float32),
                         (BL, D_MODEL)).copy()
    ls = np.broadcast_to(ln_scale, (BL, D_MODEL)).copy()
    lb = np.broadcast_to(ln_bias, (BL, D_MODEL)).copy()

    in_maps = []
    for c in range(NCORES):
        z_loc = np.ascontiguousarray(z[c * BL:(c + 1) * BL])
        zt_loc = np.ascontiguousarray(z_loc.T)
        poolT_loc = np.ascontiguousarray(
            pool[c * NL:(c + 1) * NL].T).astype(np_keys)
        in_maps.append({
            "z": z_loc, "zt": zt_loc, "poolT": poolT_loc,
            "wk": wk_cat, "wq": wq_cat, "vt": vt, "up": up, "pb": pb,
            "wbt": wbt, "gb": gb, "ls": ls, "lb": lb,
        })

    res = run_bass_kernel_spmd(nc, in_maps, core_ids=list(range(NCORES)),
                               trace=TRACE)
    LAST_EXEC_NS = res.exec_time_ns
    out = np.concatenate([res.results[c]["out"] for c in range(NCORES)],
                         axis=0)
    return out.astype(np.float32)



# revision 4
# speedup vs baseline: 358.0287x; 358.0287x over previous
"""DWA LanguageModel layer on 8 trn2 NeuronCores.

Strategy:
  - Tokens (B=1024) data-parallel across 8 cores (128 each).
  - Keys (pool @ W_K, token-independent) N-sharded: each core computes
    normalized+aspect-weighted keys for its 128 pool rows, AllGather.
  - Low-rank update never materializes UV: h_delta[b] = sum_n alpha[b,n]
    U_n (V_n z_b) via dense matmuls over nr=(n,r) with alpha sparsity
    handled by top-16 masking (masked alpha entries are exactly 0).
  - Top-16 threshold per token via vector.max + match_replace + vector.max.
  - alpha normalization deferred to the final combine (linear in alpha).
"""
import sys

sys.path.insert(0, "/opt/trn_rl_repo")
import numpy as np
import ml_dtypes

import concourse.bass as bass
import concourse.mybir as mybir
import concourse.tile as tile
from concourse import bacc
from concourse.bass_utils import run_bass_kernel_spmd
from concourse.masks import make_identity

F32 = mybir.dt.float32
BF16 = mybir.dt.bfloat16
AF = mybir.ActivationFunctionType
ALU = mybir.AluOpType

NCORES = 8
B = 1024            # tokens
BL = B // NCORES    # tokens per core = 128
D_MODEL = 512
N = 1024            # pool rows
D = 16384           # pool cols
S = 2
DK = 64
SDK = S * DK        # 128
R = 8
NR = N * R          # 8192
K_MAX = 16
LAMBDA_SHARP = 5.0
LN_EPS = 1e-5
U_END = D_MODEL * R          # 4096
V_END = U_END + R * D_MODEL  # 8192
B_END = V_END + D_MODEL      # 8704
NL = N // NCORES             # pool rows per core = 128

# dtype config: keys path and dynamic (post-alpha) path
DT_KEYS = F32
DT_DYN = BF16

LAST_EXEC_NS = None
TRACE = False
TMPDIR = None
NO_CC = False
LEVEL = 9  # bisect: 0=io 1=keys 2=+AG 3=+scores 4=alpha 5=s 6=no-bias 9=full


def _np_dt(dt):
    return {F32: np.float32, BF16: ml_dtypes.bfloat16}[dt]


def _build(tau_f, w0_f, w1_f, gamma_f):
    nc = bacc.Bacc("TRN2", target_bir_lowering=False, debug=False,
                   num_devices=NCORES)

    # ---- I/O ----
    z_d = nc.dram_tensor("z", [BL, D_MODEL], F32, kind="ExternalInput")
    zt_d = nc.dram_tensor("zt", [D_MODEL, BL], F32, kind="ExternalInput")
    poolT_d = nc.dram_tensor("poolT", [D, NL], DT_KEYS, kind="ExternalInput")
    wk_d = nc.dram_tensor("wk", [D, SDK], DT_KEYS, kind="ExternalInput")
    wq_d = nc.dram_tensor("wq", [D_MODEL, SDK], F32, kind="ExternalInput")
    vt_d = nc.dram_tensor("vt", [D_MODEL, NR], DT_DYN, kind="ExternalInput")
    up_d = nc.dram_tensor("up", [NR, D_MODEL], DT_DYN, kind="ExternalInput")
    pb_d = nc.dram_tensor("pb", [N, D_MODEL], DT_DYN, kind="ExternalInput")
    wbt_d = nc.dram_tensor("wbt", [D_MODEL, D_MODEL], F32, kind="ExternalInput")
    gb_d = nc.dram_tensor("gb", [BL, D_MODEL], F32, kind="ExternalInput")
    ls_d = nc.dram_tensor("ls", [BL, D_MODEL], F32, kind="ExternalInput")
    lb_d = nc.dram_tensor("lb", [BL, D_MODEL], F32, kind="ExternalInput")
    out_d = nc.dram_tensor("out", [BL, D_MODEL], F32, kind="ExternalOutput")

    with tile.TileContext(nc) as tc:
        with (
            tc.tile_pool(name="sb", bufs=1) as sb,            # persistent tiles
            tc.tile_pool(name="sbs", bufs=4) as sbs,          # streamed tiles
            tc.tile_pool(name="dram", bufs=1, space="DRAM") as dram,
        ):
            _emit(nc, tc, sb, sbs, dram, locals(), tau_f, w0_f, w1_f, gamma_f,
                  z_d, zt_d, poolT_d, wk_d, wq_d, vt_d, up_d, pb_d, wbt_d,
                  gb_d, ls_d, lb_d, out_d)

    nc.compile()
    return nc


def _emit(nc, tc, sb, sbs, dram, _l, tau_f, w0_f, w1_f, gamma_f,
          z_d, zt_d, poolT_d, wk_d, wq_d, vt_d, up_d, pb_d, wbt_d,
          gb_d, ls_d, lb_d, out_d):
    if LEVEL <= 0:
        z0 = sb.tile([BL, D_MODEL], F32, tag="z0")
        nc.sync.dma_start(z0[:], z_d[:])
        pt0 = sb.tile([128, NL], DT_KEYS, tag="pt0")
        nc.sync.dma_start(pt0[:], poolT_d[0:128, :])
        vt0 = sb.tile([128, 512], DT_DYN, tag="vt0")
        nc.sync.dma_start(vt0[:], vt_d[0:128, 0:512])
        up0 = sb.tile([128, 512], DT_DYN, tag="up0")
        nc.sync.dma_start(up0[:], up_d[0:128, :])
        o0 = sb.tile([BL, D_MODEL], F32, tag="o0")
        nc.vector.tensor_scalar_mul(o0[:], z0[:], 2.0)
        nc.sync.dma_start(out_d[:], o0[:])
        return

    # ---------- small persistent loads ----------
    ident = sb.tile([128, 128], F32, tag="ident")
    make_identity(nc, ident[:])
    if DT_DYN != F32:
        identd = sb.tile([128, 128], DT_DYN, tag="identd")
        make_identity(nc, identd[:])
    else:
        identd = ident
    z_sb = sb.tile([BL, D_MODEL], F32, tag="z")
    nc.sync.dma_start(z_sb[:], z_d[:])
    gb_sb = sb.tile([BL, D_MODEL], F32, tag="gb")
    nc.sync.dma_start(gb_sb[:], gb_d[:])
    ls_sb = sb.tile([BL, D_MODEL], F32, tag="ls")
    nc.sync.dma_start(ls_sb[:], ls_d[:])
    lb_sb = sb.tile([BL, D_MODEL], F32, tag="lb")
    nc.sync.dma_start(lb_sb[:], lb_d[:])
    # zb = z + gamma * b_base   (gb = gamma*b_base replicated)
    zb_sb = sb.tile([BL, D_MODEL], F32, tag="zb")
    nc.vector.tensor_add(zb_sb[:], z_sb[:], gb_sb[:])

    # zt: [128, 4*128], chunk c holds rows a=c*128..(c+1)*128 of z^T
    zt_sb = sb.tile([128, D_MODEL], F32, tag="zt")
    for c in range(4):
        nc.sync.dma_start(zt_sb[:, c * 128:(c + 1) * 128],
                          zt_d[c * 128:(c + 1) * 128, :])
    wq_sb = sb.tile([128, 4 * SDK], F32, tag="wq")
    for c in range(4):
        nc.sync.dma_start(wq_sb[:, c * SDK:(c + 1) * SDK],
                          wq_d[c * 128:(c + 1) * 128, :])
    if DT_DYN != F32:
        ztd_sb = sb.tile([128, D_MODEL], DT_DYN, tag="ztd")
        nc.vector.tensor_copy(ztd_sb[:], zt_sb[:])
    else:
        ztd_sb = zt_sb

    # ---------- phase A: keys (N-sharded) ----------
    with tc.tile_pool(name="psA", bufs=1, space="PSUM") as psA:
        keys_ps = psA.tile([NL, SDK], F32, tag="keys")
        KB = 16  # D-chunks per DMA batch
        nblk = D // (128 * KB)
        for blk in range(nblk):
            lo, hi = blk * 128 * KB, (blk + 1) * 128 * KB
            pt = sbs.tile([128, KB * NL], DT_KEYS, tag="pt")
            nc.sync.dma_start(
                pt[:].rearrange("p (i n) -> p i n", i=KB),
                poolT_d[lo:hi, :].rearrange("(i p) n -> p i n", p=128))
            wkt = sbs.tile([128, KB * SDK], DT_KEYS, tag="wkt")
            nc.sync.dma_start(
                wkt[:].rearrange("p (i n) -> p i n", i=KB),
                wk_d[lo:hi, :].rearrange("(i p) n -> p i n", p=128))
            for i in range(KB):
                d = blk * KB + i
                nc.tensor.matmul(keys_ps[:],
                                 pt[:, i * NL:(i + 1) * NL],
                                 wkt[:, i * SDK:(i + 1) * SDK],
                                 start=(d == 0), stop=(d == D // 128 - 1))
        # normalize along free (two 64-wide aspect groups), fold w_s
        ksq = sb.tile([NL, S], F32, tag="ksq")
        ksc = sb.tile([NL, SDK], F32, tag="ksc")  # scratch square out
        for s in range(S):
            nc.scalar.activation(ksc[:, s * DK:(s + 1) * DK],
                                 keys_ps[:, s * DK:(s + 1) * DK],
                                 AF.Square,
                                 accum_out=ksq[:, s:s + 1])
        knorm = sb.tile([NL, S], F32, tag="knorm")
        nc.scalar.activation(knorm[:], ksq[:], AF.Sqrt)
        nc.vector.tensor_scalar_add(knorm[:], knorm[:], 1e-8)
        krec = sb.tile([NL, S], F32, tag="krec")
        nc.vector.reciprocal(krec[:], knorm[:])
        kn_w = sb.tile([NL, SDK], F32, tag="kn_w")
        for s, w_s in ((0, w0_f), (1, w1_f)):
            nc.vector.tensor_scalar(
                kn_w[:, s * DK:(s + 1) * DK],
                keys_ps[:, s * DK:(s + 1) * DK],
                krec[:, s:s + 1], float(w_s),
                op0=ALU.mult, op1=ALU.mult)

        if LEVEL <= 1:
            nc.sync.dma_start(out_d[:, :SDK], kn_w[:])
            return

        # ---------- phase B: queries ----------
        q_ps = psA.tile([BL, SDK], F32, tag="q")
        for c in range(4):
            nc.tensor.matmul(q_ps[:],
                             zt_sb[:, c * 128:(c + 1) * 128],
                             wq_sb[:, c * SDK:(c + 1) * SDK],
                             start=(c == 0), stop=(c == 3))
        qsq = sb.tile([BL, S], F32, tag="qsq")
        qsc = sb.tile([BL, SDK], F32, tag="qsc")
        for s in range(S):
            nc.scalar.activation(qsc[:, s * DK:(s + 1) * DK],
                                 q_ps[:, s * DK:(s + 1) * DK],
                                 AF.Square,
                                 accum_out=qsq[:, s:s + 1])
        qnorm = sb.tile([BL, S], F32, tag="qnorm")
        nc.scalar.activation(qnorm[:], qsq[:], AF.Sqrt)
        nc.vector.tensor_scalar_add(qnorm[:], qnorm[:], 1e-8)
        qrec = sb.tile([BL, S], F32, tag="qrec")
        nc.vector.reciprocal(qrec[:], qnorm[:])
        q_n = sb.tile([BL, SDK], F32, tag="q_n")
        for s in range(S):
            nc.vector.tensor_scalar(
                q_n[:, s * DK:(s + 1) * DK],
                q_ps[:, s * DK:(s + 1) * DK],
                qrec[:, s:s + 1], None, op0=ALU.mult)

    # ---------- AllGather keys ----------
    cc_in = dram.tile([NL, SDK], F32)
    cc_out = dram.tile([N, SDK], F32)
    nc.sync.dma_start(cc_in[:], kn_w[:])
    if NO_CC:
        for c in range(NCORES):
            nc.sync.dma_start(cc_out[c * NL:(c + 1) * NL, :], cc_in[:])
    else:
        nc.gpsimd.collective_compute(
            "AllGather", ALU.bypass,
            replica_groups=[list(range(NCORES))],
            ins=[cc_in[:].opt()], outs=[cc_out[:].opt()],
        )

    if LEVEL <= 2:
        gat = sb.tile([NL, SDK], F32, tag="gat")
        nc.sync.dma_start(gat[:], cc_out[0:NL, :])
        nc.sync.dma_start(out_d[:, :SDK], gat[:])
        return

    with tc.tile_pool(name="psC", bufs=2, space="PSUM") as psC:
        # transpose q_n -> qnT [sdk, b]
        qn_tp = psC.tile([SDK, BL], F32, tag="ktr")
        nc.tensor.transpose(qn_tp[:], q_n[:], ident[:])
        qnT = sb.tile([SDK, BL], F32, tag="qnT")
        nc.scalar.activation(qnT[:], qn_tp[:], AF.Copy)

        # load gathered keys, transpose to knT [sdk, n]
        knT = sb.tile([SDK, N], F32, tag="knT")
        for c in range(NCORES):
            kc = sbs.tile([NL, SDK], F32, tag="kc")
            nc.sync.dma_start(kc[:], cc_out[c * NL:(c + 1) * NL, :])
            ktp = psC.tile([SDK, NL], F32, tag="ktr")
            nc.tensor.transpose(ktp[:], kc[:], ident[:])
            nc.scalar.activation(knT[:, c * NL:(c + 1) * NL],
                                 ktp[:], AF.Copy)

        # ---------- scores [b, n] ----------
        # sum over both aspects = single K=128 contraction (w folded in keys)
        scores = sb.tile([BL, N], F32, tag="scores")
        for j in range(2):
            sc_ps = psC.tile([BL, 512], F32, tag="sc")
            nc.tensor.matmul(sc_ps[:], qnT[:],
                             knT[:, j * 512:(j + 1) * 512],
                             start=True, stop=True)
            nc.scalar.activation(scores[:, j * 512:(j + 1) * 512],
                                 sc_ps[:], AF.Copy)

    if LEVEL <= 3:
        nc.sync.dma_start(out_d[:], scores[:, :D_MODEL])
        return

    # ---------- top-16 threshold + alpha ----------
    m8a = sb.tile([BL, 8], F32, tag="m8a")
    nc.vector.max(out=m8a[:], in_=scores[:])
    s_mr = sb.tile([BL, N], F32, tag="s_mr")
    nc.vector.match_replace(out=s_mr[:], in_to_replace=m8a[:],
                            in_values=scores[:], imm_value=-1e30)
    m8b = sb.tile([BL, 8], F32, tag="m8b")
    nc.vector.max(out=m8b[:], in_=s_mr[:])
    # threshold = 16th largest = m8b[:, 7]
    sig_b = sb.tile([BL, 1], F32, tag="sig_b")
    nc.vector.memset(sig_b[:], float(-LAMBDA_SHARP * tau_f))
    sig = sb.tile([BL, N], F32, tag="sig")
    nc.scalar.activation(sig[:], scores[:], AF.Sigmoid,
                         scale=LAMBDA_SHARP, bias=sig_b[:])
    ex = sb.tile([BL, N], F32, tag="ex")
    nc.scalar.activation(ex[:], scores[:], AF.Exp)
    ge = sb.tile([BL, N], F32, tag="ge")
    nc.vector.tensor_mul(ge[:], sig[:], ex[:])
    alpha = sb.tile([BL, N], F32, tag="alpha")
    den = sb.tile([BL, 1], F32, tag="den")
    nc.vector.scalar_tensor_tensor(
        out=alpha[:], in0=scores[:], scalar=m8b[:, 7:8], in1=ge[:],
        op0=ALU.is_ge, op1=ALU.mult, accum_out=den[:])
    nc.vector.tensor_scalar_add(den[:], den[:], 1e-8)
    rden = sb.tile([BL, 1], F32, tag="rden")
    nc.vector.reciprocal(rden[:], den[:])

    if LEVEL <= 4:
        nc.sync.dma_start(out_d[:], alpha[:, :D_MODEL])
        return

    s_sb = sb.tile([BL, NR], DT_DYN, tag="s_sb")

    with tc.tile_pool(name="psE", bufs=1, space="PSUM") as psE:
        # ---------- t = z @ V^T, s = alpha * t ----------
        for j in range(16):
            t_ps = psE.tile([BL, 512], F32, tag="t")
            vtt = sbs.tile([128, 4 * 512], DT_DYN, tag="vtt")
            nc.sync.dma_start(
                vtt[:].rearrange("p (c f) -> p c f", c=4),
                vt_d[:, j * 512:(j + 1) * 512]
                .rearrange("(c p) f -> p c f", p=128))
            for c in range(4):
                nc.tensor.matmul(t_ps[:],
                                 ztd_sb[:, c * 128:(c + 1) * 128],
                                 vtt[:, c * 512:(c + 1) * 512],
                                 start=(c == 0), stop=(c == 3))
            nc.vector.tensor_tensor(
                out=s_sb[:, j * 512:(j + 1) * 512]
                    .rearrange("p (n r) -> p n r", r=R),
                in0=t_ps[:].rearrange("p (n r) -> p n r", r=R),
                in1=alpha[:, j * 64:(j + 1) * 64]
                    .unsqueeze(2).broadcast_to([BL, 64, R]),
                op=ALU.mult)

        if LEVEL <= 5:
            nc.gpsimd.dma_start(out_d[:], s_sb[:, :D_MODEL])
            return

        # ---------- h1 = sT @ U (+ alphaT @ bias), unnormalized ----
        h1_ps = psE.tile([BL, D_MODEL], F32, tag="h1")
        for gb in range(16):
            upt = sbs.tile([128, 4 * D_MODEL], DT_DYN, tag="upt")
            nc.sync.dma_start(
                upt[:].rearrange("p (i c) -> p i c", i=4),
                up_d[gb * 512:(gb + 1) * 512, :]
                .rearrange("(i p) c -> p i c", p=128))
            for i in range(4):
                g = gb * 4 + i
                st_tp = psE.tile([128, 128], DT_DYN, tag="tr")
                nc.tensor.transpose(st_tp[:],
                                    s_sb[:, g * 128:(g + 1) * 128],
                                    identd[:])
                sT = sbs.tile([128, 128], DT_DYN, tag="sT")
                nc.scalar.activation(sT[:], st_tp[:], AF.Copy)
                nc.tensor.matmul(h1_ps[:], sT[:],
                                 upt[:, i * D_MODEL:(i + 1) * D_MODEL],
                                 start=(g == 0), stop=(LEVEL <= 6 and g == 63))
        if LEVEL >= 7:
            for c in range(NCORES):
                al_tp = psE.tile([128, 128], F32, tag="atr")
                nc.tensor.transpose(al_tp[:],
                                    alpha[:, c * 128:(c + 1) * 128],
                                    ident[:])
                aT = sbs.tile([128, 128], DT_DYN, tag="aT")
                nc.scalar.activation(aT[:], al_tp[:], AF.Copy)
                pbt = sbs.tile([128, D_MODEL], DT_DYN, tag="pbt")
                nc.sync.dma_start(pbt[:], pb_d[c * NL:(c + 1) * NL, :])
                nc.tensor.matmul(h1_ps[:], aT[:], pbt[:],
                                 start=False, stop=(c == NCORES - 1))

        # ---------- h2 = z @ W_base^T ----------
        h2_ps = psE.tile([BL, D_MODEL], F32, tag="h2")
        for c in range(4):
            wbt = sbs.tile([128, D_MODEL], F32, tag="wbt")
            nc.sync.dma_start(wbt[:], wbt_d[c * 128:(c + 1) * 128, :])
            nc.tensor.matmul(h2_ps[:],
                             zt_sb[:, c * 128:(c + 1) * 128],
                             wbt[:], start=(c == 0), stop=(c == 3))

        # ---------- combine ----------
        A_sb = sb.tile([BL, D_MODEL], F32, tag="A")
        nc.vector.tensor_scalar(A_sb[:], h1_ps[:], rden[:], None,
                                op0=ALU.mult)
        nc.vector.tensor_add(A_sb[:], A_sb[:], h2_ps[:])

    # ---------- layernorm ----------
    x_sb = sb.tile([BL, D_MODEL], F32, tag="x")
    nc.vector.scalar_tensor_tensor(
        out=x_sb[:], in0=A_sb[:], scalar=float(gamma_f), in1=zb_sb[:],
        op0=ALU.mult, op1=ALU.add)
    mean = sb.tile([BL, 1], F32, tag="mean")
    nc.vector.reduce_sum(mean[:], x_sb[:], axis=mybir.AxisListType.X)
    nc.vector.tensor_scalar_mul(mean[:], mean[:], 1.0 / D_MODEL)
    xc = sb.tile([BL, D_MODEL], F32, tag="xc")
    nc.vector.tensor_scalar(xc[:], x_sb[:], mean[:], None,
                            op0=ALU.subtract)
    xsq = sb.tile([BL, D_MODEL], F32, tag="xsq")
    ssq = sb.tile([BL, 1], F32, tag="ssq")
    nc.scalar.activation(xsq[:], xc[:], AF.Square, accum_out=ssq[:])
    vare = sb.tile([BL, 1], F32, tag="vare")
    nc.vector.tensor_scalar(vare[:], ssq[:], 1.0 / D_MODEL, LN_EPS,
                            op0=ALU.mult, op1=ALU.add)
    sd = sb.tile([BL, 1], F32, tag="sd")
    nc.scalar.activation(sd[:], vare[:], AF.Sqrt)
    rstd = sb.tile([BL, 1], F32, tag="rstd")
    nc.vector.reciprocal(rstd[:], sd[:])
    y1 = sb.tile([BL, D_MODEL], F32, tag="y1")
    nc.vector.tensor_scalar(y1[:], xc[:], rstd[:], None, op0=ALU.mult)
    out_sb = sb.tile([BL, D_MODEL], F32, tag="out_sb")
    nc.vector.tensor_mul(out_sb[:], y1[:], ls_sb[:])
    nc.vector.tensor_add(out_sb[:], out_sb[:], lb_sb[:])
    nc.sync.dma_start(out_d[:], out_sb[:])


def kernel(z, pool_vectors, W_Q, W_K, aspect_logits, tau,
           W_base, b_base, gamma, ln_scale, ln_bias):
    global LAST_EXEC_NS
    z = np.asarray(z, np.float32)
    pool = np.asarray(pool_vectors, np.float32)
    W_Q = np.asarray(W_Q, np.float32)
    W_K = np.asarray(W_K, np.float32)
    aspect_logits = np.asarray(aspect_logits, np.float32)
    tau_f = float(np.asarray(tau))
    W_base = np.asarray(W_base, np.float32)
    b_base = np.asarray(b_base, np.float32)
    gamma_f = float(np.asarray(gamma))
    ln_scale = np.asarray(ln_scale, np.float32)
    ln_bias = np.asarray(ln_bias, np.float32)

    e = np.exp(aspect_logits - aspect_logits.max())
    w = e / e.sum()
    w0_f, w1_f = float(w[0]), float(w[1])

    nc = _build(tau_f, w0_f, w1_f, gamma_f)

    # ---- host-side layout prep ----
    np_keys = _np_dt(DT_KEYS)
    np_dyn = _np_dt(DT_DYN)
    wk_cat = np.concatenate([W_K[0], W_K[1]], axis=1).astype(np_keys)  # [D, 128]
    wq_cat = np.concatenate([W_Q[0], W_Q[1]], axis=1).astype(np.float32)
    # V^T: [e, n*R + r] = pool[n, 4096 + r*512 + e]
    vt = np.ascontiguousarray(
        pool[:, U_END:V_END].reshape(N, R, D_MODEL).transpose(2, 0, 1)
        .reshape(D_MODEL, NR)).astype(np_dyn)
    # U_perm: [n*R + r, c] = pool[n, c*R + r]
    up = np.ascontiguousarray(
        pool[:, :U_END].reshape(N, D_MODEL, R).transpose(0, 2, 1)
        .reshape(NR, D_MODEL)).astype(np_dyn)
    pb = np.ascontiguousarray(pool[:, V_END:B_END]).astype(np_dyn)  # [N, 512]
    wbt = np.ascontiguousarray(W_base.T)  # [a, c]
    gb = np.broadcast_to((gamma_f * b_base).astype(np.float32),
                         (BL, D_MODEL)).copy()
    ls = np.broadcast_to(ln_scale, (BL, D_MODEL)).copy()
    lb = np.broadcast_to(ln_bias, (BL, D_MODEL)).copy()

    in_maps = []
    for c in range(NCORES):
        z_loc = np.ascontiguousarray(z[c * BL:(c + 1) * BL])
        zt_loc = np.ascontiguousarray(z_loc.T)
        poolT_loc = np.ascontiguousarray(
            pool[c * NL:(c + 1) * NL].T).astype(np_keys)
        in_maps.append({
            "z": z_loc, "zt": zt_loc, "poolT": poolT_loc,
            "wk": wk_cat, "wq": wq_cat, "vt": vt, "up": up, "pb": pb,
            "wbt": wbt, "gb": gb, "ls": ls, "lb": lb,
        })

    res = run_bass_kernel_spmd(nc, in_maps, core_ids=list(range(NCORES)),
                               trace=TRACE, tmpdir=TMPDIR)
    LAST_EXEC_NS = res.exec_time_ns
    out = np.concatenate([res.results[c]["out"] for c in range(NCORES)],
                         axis=0)
    return out.astype(np.float32)



# revision 12
# speedup vs baseline: 545.2936x; 1.5230x over previous
"""DWA LanguageModel layer on 8 trn2 NeuronCores (v2).

Strategy:
  - Tokens (B=1024) data-parallel across 8 cores (128 each).
  - Keys D-sharded: each core contracts its D/8=2048 slice of
    pool^T @ W_K for ALL N=1024 pool rows, producing keysT [sdk, N]
    directly in the scores-ready layout; one 512KB AllReduce sums the
    partials. Key norms (reduction over sdk = partition dim) via two
    tiny indicator matmuls.
  - Dynamic path fp8 (e4m3, scaled): never materializes UV;
    h_delta[b] = sum_nr s[b,nr] U[nr,:] with s = alpha * (z @ V^T).
    Pool bias folded into the same contraction as 8 extra chunks
    (s' = [s, 16*alpha], up' = [64*U_perm, 64*bias]); the 1024x scale
    is folded into the alpha-normalization reciprocal.
  - Top-16 threshold per token via vector.max + match_replace + max.
  - All large DMAs are single [128, X] contiguous transfers (host
    pre-packs the exact SBUF layout).
"""
import sys

sys.path.insert(0, "/opt/trn_rl_repo")
import numpy as np
import ml_dtypes

import concourse.bass as bass
import concourse.mybir as mybir
import concourse.tile as tile
from concourse import bacc
from concourse.bass_utils import run_bass_kernel_spmd
from concourse.masks import make_identity

F32 = mybir.dt.float32
BF16 = mybir.dt.bfloat16
FP8 = mybir.dt.float8e4
AF = mybir.ActivationFunctionType
ALU = mybir.AluOpType

NCORES = 8
B = 1024            # tokens
BL = B // NCORES    # tokens per core = 128
D_MODEL = 512
N = 1024            # pool rows
D = 16384           # pool cols
DL = D // NCORES    # pool cols per core = 2048
S = 2
DK = 64
SDK = S * DK        # 128
R = 8
NR = N * R          # 8192
K_MAX = 16
LAMBDA_SHARP = 5.0
LN_EPS = 1e-5
U_END = D_MODEL * R          # 4096
V_END = U_END + R * D_MODEL  # 8192
B_END = V_END + D_MODEL      # 8704

SC_V = 16.0          # scale on V^T (and on alpha chunk of s')
SC_U = 64.0          # scale on U_perm / bias
SC_H = SC_V * SC_U   # total scale on h1 psum = 1024

LAST_EXEC_NS = None
TRACE = False
TMPDIR = None
NO_CC = False
LEVEL = 9  # bisect: 0=io 3=scores 4=alpha 9=full


def _build(tau_f, w0_f, w1_f, gamma_f):
    nc = bacc.Bacc("TRN2", target_bir_lowering=False, debug=False,
                   num_devices=NCORES)

    # ---- I/O (all pre-packed to exact SBUF layout [128, X]) ----
    pk_d = nc.dram_tensor("pk", [128, 16 * N], BF16, kind="ExternalInput")
    wk_d = nc.dram_tensor("wk", [128, 16 * SDK], BF16, kind="ExternalInput")
    wq_d = nc.dram_tensor("wq", [128, 4 * SDK], F32, kind="ExternalInput")
    zt_d = nc.dram_tensor("zt", [128, 4 * BL], F32, kind="ExternalInput")
    zb_d = nc.dram_tensor("zb", [BL, D_MODEL], F32, kind="ExternalInput")
    ls_d = nc.dram_tensor("ls", [BL, D_MODEL], F32, kind="ExternalInput")
    lb_d = nc.dram_tensor("lb", [BL, D_MODEL], F32, kind="ExternalInput")
    wbt_d = nc.dram_tensor("wbt", [128, 4 * D_MODEL], BF16,
                           kind="ExternalInput")
    vt_d = nc.dram_tensor("vt", [128, 4 * NR], FP8, kind="ExternalInput")
    up_d = nc.dram_tensor("up", [128, 72 * D_MODEL], FP8,
                          kind="ExternalInput")
    e2_d = nc.dram_tensor("e2", [128, S], F32, kind="ExternalInput")
    e2t_d = nc.dram_tensor("e2t", [S, 128], F32, kind="ExternalInput")
    out_d = nc.dram_tensor("out", [BL, D_MODEL], F32, kind="ExternalOutput")

    with tile.TileContext(nc) as tc:
        with (
            tc.tile_pool(name="sb", bufs=1) as sb,
            tc.tile_pool(name="sbr", bufs=3) as sbr,     # rotating sT tiles
            tc.tile_pool(name="dram", bufs=1, space="DRAM") as dram,
        ):
            _emit(nc, tc, sb, sbr, dram, tau_f, w0_f, w1_f, gamma_f,
                  pk_d, wk_d, wq_d, zt_d, zb_d, ls_d, lb_d, wbt_d,
                  vt_d, up_d, e2_d, e2t_d, out_d)

    nc.compile()
    return nc


def _emit(nc, tc, sb, sbr, dram, tau_f, w0_f, w1_f, gamma_f,
          pk_d, wk_d, wq_d, zt_d, zb_d, ls_d, lb_d, wbt_d,
          vt_d, up_d, e2_d, e2t_d, out_d):
    # ---------- DMA loads, priority order ----------
    pk_sb = sb.tile([128, 16 * N], BF16, tag="pk")
    nc.sync.dma_start(pk_sb[:], pk_d[:])
    wk_sb = sb.tile([128, 16 * SDK], BF16, tag="wk")
    nc.sync.dma_start(wk_sb[:], wk_d[:])
    zt_sb = sb.tile([128, 4 * BL], F32, tag="zt")
    nc.sync.dma_start(zt_sb[:], zt_d[:])
    wq_sb = sb.tile([128, 4 * SDK], F32, tag="wq")
    nc.sync.dma_start(wq_sb[:], wq_d[:])
    zb_sb = sb.tile([BL, D_MODEL], F32, tag="zb")
    nc.sync.dma_start(zb_sb[:], zb_d[:])
    wbt_sb = sb.tile([128, 4 * D_MODEL], BF16, tag="wbt")
    nc.sync.dma_start(wbt_sb[:], wbt_d[:])
    vt_sb = sb.tile([128, 4 * NR], FP8, tag="vt")
    nc.sync.dma_start(vt_sb[:], vt_d[:])
    up_sb = sb.tile([128, 72 * D_MODEL], FP8, tag="up")
    nc.sync.dma_start(up_sb[:], up_d[:])
    ls_sb = sb.tile([BL, D_MODEL], F32, tag="ls")
    nc.sync.dma_start(ls_sb[:], ls_d[:])
    lb_sb = sb.tile([BL, D_MODEL], F32, tag="lb")
    nc.sync.dma_start(lb_sb[:], lb_d[:])

    if LEVEL <= 0:
        o0 = sb.tile([BL, D_MODEL], F32, tag="o0")
        nc.vector.tensor_scalar_mul(o0[:], zb_sb[:], 2.0)
        nc.sync.dma_start(out_d[:], o0[:])
        return

    # ---------- small constants ----------
    identb = sb.tile([128, 128], BF16, tag="identb")
    make_identity(nc, identb[:])
    # aspect indicator matrices for partition-dim norm reduction
    E2 = sb.tile([128, S], F32, tag="E2")
    nc.sync.dma_start(E2[:], e2_d[:])
    E2T = sb.tile([S, 128], F32, tag="E2T")
    nc.sync.dma_start(E2T[:], e2t_d[:])
    ztb = sb.tile([128, 4 * BL], BF16, tag="ztb")
    nc.vector.tensor_copy(ztb[:], zt_sb[:])

    kraw_sb = sb.tile([SDK, N], F32, tag="kraw")
    t_sb = sb.tile([BL, NR], BF16, tag="t")
    q_n = sb.tile([BL, SDK], BF16, tag="q_n")
    qnT = sb.tile([SDK, BL], BF16, tag="qnT")
    h2_sb = sb.tile([BL, D_MODEL], F32, tag="h2")

    with tc.tile_pool(name="ps1", bufs=1, space="PSUM") as ps1:
        # ---------- keysT partials [sdk, N] over local D slice ----------
        kT_ps0 = ps1.tile([SDK, 512], F32, tag="kT0")
        kT_ps1 = ps1.tile([SDK, 512], F32, tag="kT1")
        for k in range(16):
            for h, kt in ((0, kT_ps0), (1, kT_ps1)):
                nc.tensor.matmul(kt[:],
                                 wk_sb[:, k * SDK:(k + 1) * SDK],
                                 pk_sb[:, k * N + h * 512:k * N + (h + 1) * 512],
                                 start=(k == 0), stop=(k == 15))
        nc.scalar.activation(kraw_sb[:, 0:512], kT_ps0[:], AF.Copy)
        nc.scalar.activation(kraw_sb[:, 512:1024], kT_ps1[:], AF.Copy)

        # ---------- queries [b, sdk] + normalize ----------
        q_ps = ps1.tile([BL, SDK], F32, tag="q")
        for c in range(4):
            nc.tensor.matmul(q_ps[:],
                             zt_sb[:, c * BL:(c + 1) * BL],
                             wq_sb[:, c * SDK:(c + 1) * SDK],
                             start=(c == 0), stop=(c == 3))
        qsq = sb.tile([BL, S], F32, tag="qsq")
        qsc = sb.tile([BL, SDK], F32, tag="qsc")
        for s in range(S):
            nc.scalar.activation(qsc[:, s * DK:(s + 1) * DK],
                                 q_ps[:, s * DK:(s + 1) * DK],
                                 AF.Square,
                                 accum_out=qsq[:, s:s + 1])
        qnorm = sb.tile([BL, S], F32, tag="qnorm")
        nc.scalar.activation(qnorm[:], qsq[:], AF.Sqrt)
        nc.vector.tensor_scalar_add(qnorm[:], qnorm[:], 1e-8)
        qrec = sb.tile([BL, S], F32, tag="qrec")
        nc.vector.reciprocal(qrec[:], qnorm[:])
        # fold softmax(aspect_logits) weights into q_n
        for s, w_s in ((0, w0_f), (1, w1_f)):
            nc.vector.tensor_scalar(
                q_n[:, s * DK:(s + 1) * DK],
                q_ps[:, s * DK:(s + 1) * DK],
                qrec[:, s:s + 1], float(w_s),
                op0=ALU.mult, op1=ALU.mult)
        # ---------- AllReduce keysT partials ----------
        cc_in = dram.tile([SDK, N], F32)
        cc_out = dram.tile([SDK, N], F32)
        nc.sync.dma_start(cc_in[:], kraw_sb[:])
        if NO_CC:
            nc.sync.dma_start(cc_out[:], cc_in[:])
        else:
            nc.gpsimd.collective_compute(
                "AllReduce", ALU.add,
                replica_groups=[list(range(NCORES))],
                ins=[cc_in[:].opt()], outs=[cc_out[:].opt()],
            )

        # ---------- t = z @ V'^T  [b, nr] (independent of CC) ----------
        with tc.tile_pool(name="psT", bufs=4, space="PSUM") as psT:
            for j in range(16):
                t_ps = psT.tile([BL, 512], F32, tag="t")
                for a in range(4):
                    nc.tensor.matmul(
                        t_ps[:],
                        ztb[:, a * BL:(a + 1) * BL],
                        vt_sb[:, a * NR + j * 512:a * NR + (j + 1) * 512],
                        start=(a == 0), stop=(a == 3))
                nc.scalar.activation(t_sb[:, j * 512:(j + 1) * 512],
                                     t_ps[:], AF.Copy)

        # ---------- h2 = z @ W_base^T (independent of CC) ----------
        h2_ps = ps1.tile([BL, D_MODEL], F32, tag="h2p")
        for c in range(4):
            nc.tensor.matmul(h2_ps[:],
                             ztb[:, c * BL:(c + 1) * BL],
                             wbt_sb[:, c * D_MODEL:(c + 1) * D_MODEL],
                             start=(c == 0), stop=(c == 3))
        nc.scalar.activation(h2_sb[:], h2_ps[:], AF.Copy)

    # ---------- key norms + scores (after CC) ----------
    keysT_sb = sb.tile([SDK, N], F32, tag="keysT")
    nc.sync.dma_start(keysT_sb[:], cc_out[:])
    scores = sb.tile([BL, N], F32, tag="scores")
    with tc.tile_pool(name="psS", bufs=2, space="PSUM") as psS:
        q_tp = psS.tile([SDK, BL], BF16, tag="qtp")
        nc.tensor.transpose(q_tp[:], q_n[:], identb[:])
        nc.scalar.activation(qnT[:], q_tp[:], AF.Copy)
        sq_sb = kraw_sb  # kraw already consumed by the CC input DMA
        nc.scalar.activation(sq_sb[:], keysT_sb[:], AF.Square)
        nsq_sb = sb.tile([S, N], F32, tag="nsq")
        for h in range(2):
            nsq_ps = psS.tile([S, 512], F32, tag="nsq")
            nc.tensor.matmul(nsq_ps[:], E2[:],
                             sq_sb[:, h * 512:(h + 1) * 512],
                             start=True, stop=True)
            nc.scalar.activation(nsq_sb[:, h * 512:(h + 1) * 512],
                                 nsq_ps[:], AF.Sqrt)
        nc.vector.tensor_scalar_add(nsq_sb[:], nsq_sb[:], 1e-8)
        rec_sb = sb.tile([S, N], F32, tag="rec")
        nc.vector.reciprocal(rec_sb[:], nsq_sb[:])
        kn_sb = sb.tile([SDK, N], BF16, tag="kn")
        for h in range(2):
            rf_ps = psS.tile([SDK, 512], F32, tag="rf")
            nc.tensor.matmul(rf_ps[:], E2T[:],
                             rec_sb[:, h * 512:(h + 1) * 512],
                             start=True, stop=True)
            nc.vector.tensor_tensor(
                out=kn_sb[:, h * 512:(h + 1) * 512],
                in0=keysT_sb[:, h * 512:(h + 1) * 512],
                in1=rf_ps[:], op=ALU.mult)
        for h in range(2):
            sc_ps = psS.tile([BL, 512], F32, tag="sc")
            nc.tensor.matmul(sc_ps[:], qnT[:],
                             kn_sb[:, h * 512:(h + 1) * 512],
                             start=True, stop=True)
            nc.scalar.activation(scores[:, h * 512:(h + 1) * 512],
                                 sc_ps[:], AF.Copy)

    if LEVEL <= 3:
        nc.sync.dma_start(out_d[:], scores[:, :D_MODEL])
        return

    # ---------- top-16 threshold + alpha (unnormalized) ----------
    m8a = sb.tile([BL, 8], F32, tag="m8a")
    nc.vector.max(out=m8a[:], in_=scores[:])
    s_mr = sb.tile([BL, N], F32, tag="s_mr")
    nc.vector.match_replace(out=s_mr[:], in_to_replace=m8a[:],
                            in_values=scores[:], imm_value=-1e30)
    m8b = sb.tile([BL, 8], F32, tag="m8b")
    nc.vector.max(out=m8b[:], in_=s_mr[:])
    sig_b = sb.tile([BL, 1], F32, tag="sig_b")
    nc.vector.memset(sig_b[:], float(-LAMBDA_SHARP * tau_f))
    sig = sb.tile([BL, N], F32, tag="sig")  # becomes ge-product input
    nc.scalar.activation(sig[:], scores[:], AF.Sigmoid,
                         scale=LAMBDA_SHARP, bias=sig_b[:])
    ex = sb.tile([BL, N], F32, tag="ex")
    nc.scalar.activation(ex[:], scores[:], AF.Exp)
    ge = s_mr  # s_mr dead after m8b
    nc.vector.tensor_mul(ge[:], sig[:], ex[:])
    alpha = sb.tile([BL, N], F32, tag="alpha")
    den = sb.tile([BL, 1], F32, tag="den")
    nc.vector.scalar_tensor_tensor(
        out=alpha[:], in0=scores[:], scalar=m8b[:, 7:8], in1=ge[:],
        op0=ALU.is_ge, op1=ALU.mult, accum_out=den[:])
    # rden2 = 1 / (SC_H * (den + 1e-8)) folds the fp8 scaling
    den2 = sb.tile([BL, 1], F32, tag="den2")
    nc.vector.tensor_scalar(den2[:], den[:], float(SC_H),
                            float(SC_H * 1e-8), op0=ALU.mult, op1=ALU.add)
    rden = sb.tile([BL, 1], F32, tag="rden")
    nc.vector.reciprocal(rden[:], den2[:])

    if LEVEL <= 4:
        nc.sync.dma_start(out_d[:], alpha[:, :D_MODEL])
        return

    # ---------- s' = [alpha*t, SC_V*alpha] in fp8 ----------
    s_sb = sb.tile([BL, NR + N], BF16, tag="s_sb")
    nc.vector.tensor_scalar_mul(s_sb[:, NR:NR + N], alpha[:], float(SC_V))
    for j in range(16):
        nc.vector.tensor_tensor(
            out=s_sb[:, j * 512:(j + 1) * 512]
                .rearrange("p (n r) -> p n r", r=R),
            in0=t_sb[:, j * 512:(j + 1) * 512]
                .rearrange("p (n r) -> p n r", r=R),
            in1=alpha[:, j * 64:(j + 1) * 64]
                .unsqueeze(2).broadcast_to([BL, 64, R]),
            op=ALU.mult)

    # ---------- h1 = s'^T-chunks @ up' (72 chunks, pipelined) ----------
    NG = 72
    with tc.tile_pool(name="psH", bufs=1, space="PSUM") as psH, \
         tc.tile_pool(name="psR", bufs=4, space="PSUM") as psR:
        h1_ps = psH.tile([BL, D_MODEL], F32, tag="h1")
        trs = []
        sTs = []

        def emit_transpose(g):
            tr = psR.tile([128, 128], BF16, tag="tr")
            nc.tensor.transpose(tr[:], s_sb[:, g * 128:(g + 1) * 128],
                                identb[:])
            sT = sbr.tile([128, 128], BF16, tag="sT")
            if g % 2 == 0:
                nc.scalar.activation(sT[:], tr[:], AF.Copy)
            else:
                nc.vector.tensor_copy(sT[:], tr[:])
            trs.append(tr)
            sTs.append(sT)

        emit_transpose(0)
        emit_transpose(1)
        for g in range(NG):
            if g + 2 < NG:
                emit_transpose(g + 2)
            nc.tensor.matmul(h1_ps[:], sTs[g][:],
                             up_sb[:, g * D_MODEL:(g + 1) * D_MODEL],
                             start=(g == 0), stop=(g == NG - 1))

        # ---------- combine + layernorm ----------
        A_sb = sb.tile([BL, D_MODEL], F32, tag="A")
        nc.vector.tensor_scalar(A_sb[:], h1_ps[:], rden[:], None,
                                op0=ALU.mult)
    nc.vector.tensor_add(A_sb[:], A_sb[:], h2_sb[:])
    x_sb = sb.tile([BL, D_MODEL], F32, tag="x")
    nc.vector.scalar_tensor_tensor(
        out=x_sb[:], in0=A_sb[:], scalar=float(gamma_f), in1=zb_sb[:],
        op0=ALU.mult, op1=ALU.add)
    mean = sb.tile([BL, 1], F32, tag="mean")
    nc.vector.reduce_sum(mean[:], x_sb[:], axis=mybir.AxisListType.X)
    nc.vector.tensor_scalar_mul(mean[:], mean[:], 1.0 / D_MODEL)
    xc = sb.tile([BL, D_MODEL], F32, tag="xc")
    nc.vector.tensor_scalar(xc[:], x_sb[:], mean[:], None,
                            op0=ALU.subtract)
    xsq = A_sb  # A dead after x computed
    ssq = sb.tile([BL, 1], F32, tag="ssq")
    nc.scalar.activation(xsq[:], xc[:], AF.Square, accum_out=ssq[:])
    vare = sb.tile([BL, 1], F32, tag="vare")
    nc.vector.tensor_scalar(vare[:], ssq[:], 1.0 / D_MODEL, LN_EPS,
                            op0=ALU.mult, op1=ALU.add)
    sd = sb.tile([BL, 1], F32, tag="sd")
    nc.scalar.activation(sd[:], vare[:], AF.Sqrt)
    rstd = sb.tile([BL, 1], F32, tag="rstd")
    nc.vector.reciprocal(rstd[:], sd[:])
    y1 = sb.tile([BL, D_MODEL], F32, tag="y1")
    nc.vector.tensor_scalar(y1[:], xc[:], rstd[:], None, op0=ALU.mult)
    out_sb = sb.tile([BL, D_MODEL], F32, tag="out_sb")
    nc.vector.tensor_mul(out_sb[:], y1[:], ls_sb[:])
    nc.vector.tensor_add(out_sb[:], out_sb[:], lb_sb[:])
    nc.sync.dma_start(out_d[:], out_sb[:])


def _pack(x, p=128):
    """[K*p, F] row-chunked -> [p, K*F] (chunk k at cols k*F:(k+1)*F)."""
    k = x.shape[0] // p
    return np.ascontiguousarray(
        x.reshape(k, p, -1).transpose(1, 0, 2).reshape(p, -1))


def kernel(z, pool_vectors, W_Q, W_K, aspect_logits, tau,
           W_base, b_base, gamma, ln_scale, ln_bias):
    global LAST_EXEC_NS
    z = np.asarray(z, np.float32)
    pool = np.asarray(pool_vectors, np.float32)
    W_Q = np.asarray(W_Q, np.float32)
    W_K = np.asarray(W_K, np.float32)
    aspect_logits = np.asarray(aspect_logits, np.float32)
    tau_f = float(np.asarray(tau))
    W_base = np.asarray(W_base, np.float32)
    b_base = np.asarray(b_base, np.float32)
    gamma_f = float(np.asarray(gamma))
    ln_scale = np.asarray(ln_scale, np.float32)
    ln_bias = np.asarray(ln_bias, np.float32)

    e = np.exp(aspect_logits - aspect_logits.max())
    w = e / e.sum()
    w0_f, w1_f = float(w[0]), float(w[1])

    nc = _build(tau_f, w0_f, w1_f, gamma_f)

    fp8 = ml_dtypes.float8_e4m3
    bf16 = ml_dtypes.bfloat16

    # ---- shared host-side layout prep ----
    wk_cat = np.concatenate([W_K[0], W_K[1]], axis=1)          # [D, 128]
    wq = _pack(np.concatenate([W_Q[0], W_Q[1]], axis=1))       # [128, 512] f32
    # V'^T: [a, n*R + r], scaled
    vt = _pack((pool[:, U_END:V_END].reshape(N, R, D_MODEL)
                .transpose(2, 0, 1).reshape(D_MODEL, NR)
                * SC_V).astype(fp8))
    # up' = [SC_U*U_perm; SC_U*bias] [9216, 512], scaled
    up_rows = np.concatenate([
        pool[:, :U_END].reshape(N, D_MODEL, R).transpose(0, 2, 1)
        .reshape(NR, D_MODEL),
        pool[:, V_END:B_END],
    ], axis=0) * SC_U
    up = _pack(up_rows.astype(fp8))                            # [128, 72*512]
    wbt = _pack(np.ascontiguousarray(W_base.T).astype(bf16))   # [128, 4*512]
    ls = np.broadcast_to(ln_scale, (BL, D_MODEL)).astype(np.float32).copy()
    lb = np.broadcast_to(ln_bias, (BL, D_MODEL)).astype(np.float32).copy()
    gb = (gamma_f * b_base).astype(np.float32)
    e2_np = np.zeros((128, S), np.float32)
    e2_np[0:DK, 0] = 1.0
    e2_np[DK:SDK, 1] = 1.0
    e2t_np = np.ascontiguousarray(e2_np.T)

    in_maps = []
    for c in range(NCORES):
        z_loc = np.ascontiguousarray(z[c * BL:(c + 1) * BL])
        zt_loc = _pack(np.ascontiguousarray(z_loc.T))
        pk_loc = _pack(np.ascontiguousarray(
            pool[:, c * DL:(c + 1) * DL].T).astype(bf16))
        wk_loc = _pack(np.ascontiguousarray(
            wk_cat[c * DL:(c + 1) * DL]).astype(bf16))
        in_maps.append({
            "pk": pk_loc, "wk": wk_loc, "wq": wq, "zt": zt_loc,
            "zb": z_loc + gb, "ls": ls, "lb": lb, "wbt": wbt,
            "vt": vt, "up": up, "e2": e2_np, "e2t": e2t_np,
        })

    res = run_bass_kernel_spmd(nc, in_maps, core_ids=list(range(NCORES)),
                               trace=TRACE, tmpdir=TMPDIR)
    LAST_EXEC_NS = res.exec_time_ns
    out = np.concatenate([res.results[c]["out"] for c in range(NCORES)],
                         axis=0)
    return out.astype(np.float32)


# revision 14
# speedup vs baseline: 660.7630x; 1.2118x over previous
"""DWA LanguageModel layer on 8 trn2 NeuronCores (v3).

Strategy:
  - Tokens (B=1024) data-parallel across 8 cores (128 each).
  - Keys N-sharded: each core computes keys for its 128 pool rows over
    the full D=16384 contraction, normalizes along the free dim,
    transposes, and AllGathers 32KB of bf16 normalized keysT (the
    collective is latency-bound, so ship the smallest payload).
  - Dynamic path fp8 (e4m3, scaled): never materializes UV;
    h_delta[b] = sum_nr s[b,nr] U[nr,:] with s = alpha * (z @ V^T).
    Pool bias folded into the same contraction as 8 extra chunks
    (s' = [s, 16*alpha], up' = [64*U_perm, 64*bias]); the 1024x scale
    folds into the alpha-normalization reciprocal.
  - Top-16 threshold per token via vector.max + match_replace + max.
  - All large DMAs are [128, X] contiguous transfers (host pre-packs
    the exact SBUF layout); s-mults split across DVE and GpSimd; the
    transpose+h1 loop is software-pipelined behind them.
"""
import sys

sys.path.insert(0, "/opt/trn_rl_repo")
import numpy as np
import ml_dtypes

import concourse.bass as bass
import concourse.mybir as mybir
import concourse.tile as tile
from concourse import bacc
from concourse.bass_utils import run_bass_kernel_spmd
from concourse.masks import make_identity

F32 = mybir.dt.float32
BF16 = mybir.dt.bfloat16
FP8 = mybir.dt.float8e4
AF = mybir.ActivationFunctionType
ALU = mybir.AluOpType

NCORES = 8
B = 1024            # tokens
BL = B // NCORES    # tokens per core = 128
D_MODEL = 512
N = 1024            # pool rows
NL = N // NCORES    # pool rows per core = 128
D = 16384           # pool cols
S = 2
DK = 64
SDK = S * DK        # 128
R = 8
NR = N * R          # 8192
K_MAX = 16
LAMBDA_SHARP = 5.0
LN_EPS = 1e-5
U_END = D_MODEL * R          # 4096
V_END = U_END + R * D_MODEL  # 8192
B_END = V_END + D_MODEL      # 8704

SC_V = 16.0          # scale on V^T (and on alpha chunk of s')
SC_U = 64.0          # scale on U_perm / bias
SC_H = SC_V * SC_U   # total scale on h1 psum = 1024

LAST_EXEC_NS = None
TRACE = False
TMPDIR = None
NO_CC = False
LEVEL = 9  # bisect: 0=io 3=scores 4=alpha 9=full


def _build(tau_f, w0_f, w1_f, gamma_f):
    nc = bacc.Bacc("TRN2", target_bir_lowering=False, debug=False,
                   num_devices=NCORES)

    # ---- I/O (all pre-packed to exact SBUF layout [128, X]) ----
    pk_d = nc.dram_tensor("pk", [128, 128 * NL], BF16, kind="ExternalInput")
    wk_d = nc.dram_tensor("wk", [128, 128 * SDK], BF16, kind="ExternalInput")
    wq_d = nc.dram_tensor("wq", [128, 4 * SDK], BF16, kind="ExternalInput")
    zt_d = nc.dram_tensor("zt", [128, 4 * BL], BF16, kind="ExternalInput")
    zb_d = nc.dram_tensor("zb", [BL, D_MODEL], F32, kind="ExternalInput")
    ls_d = nc.dram_tensor("ls", [BL, D_MODEL], F32, kind="ExternalInput")
    lb_d = nc.dram_tensor("lb", [BL, D_MODEL], F32, kind="ExternalInput")
    wbt_d = nc.dram_tensor("wbt", [128, 4 * D_MODEL], BF16,
                           kind="ExternalInput")
    vt_d = nc.dram_tensor("vt", [128, 4 * NR], FP8, kind="ExternalInput")
    up_d = nc.dram_tensor("up", [128, 72 * D_MODEL], FP8,
                          kind="ExternalInput")
    out_d = nc.dram_tensor("out", [BL, D_MODEL], F32, kind="ExternalOutput")

    with tile.TileContext(nc) as tc:
        with (
            tc.tile_pool(name="sb", bufs=1) as sb,
            tc.tile_pool(name="sbr", bufs=4) as sbr,     # rotating sT tiles
            tc.tile_pool(name="dram", bufs=1, space="DRAM") as dram,
        ):
            _emit(nc, tc, sb, sbr, dram, tau_f, w0_f, w1_f, gamma_f,
                  pk_d, wk_d, wq_d, zt_d, zb_d, ls_d, lb_d, wbt_d,
                  vt_d, up_d, out_d)

    nc.compile()
    return nc


def _emit(nc, tc, sb, sbr, dram, tau_f, w0_f, w1_f, gamma_f,
          pk_d, wk_d, wq_d, zt_d, zb_d, ls_d, lb_d, wbt_d,
          vt_d, up_d, out_d):
    # ---------- DMA loads, priority order (chunked for pipelining) ----------
    pk_sb = sb.tile([128, 128 * NL], BF16, tag="pk")
    wk_sb = sb.tile([128, 128 * SDK], BF16, tag="wk")
    for i in range(4):
        q4 = 32 * 128
        nc.sync.dma_start(pk_sb[:, i * q4:(i + 1) * q4],
                          pk_d[:, i * q4:(i + 1) * q4])
        nc.sync.dma_start(wk_sb[:, i * q4:(i + 1) * q4],
                          wk_d[:, i * q4:(i + 1) * q4])
    zt_sb = sb.tile([128, 4 * BL], BF16, tag="zt")
    nc.sync.dma_start(zt_sb[:], zt_d[:])
    wq_sb = sb.tile([128, 4 * SDK], BF16, tag="wq")
    nc.sync.dma_start(wq_sb[:], wq_d[:])
    zb_sb = sb.tile([BL, D_MODEL], F32, tag="zb")
    nc.sync.dma_start(zb_sb[:], zb_d[:])
    wbt_sb = sb.tile([128, 4 * D_MODEL], BF16, tag="wbt")
    nc.sync.dma_start(wbt_sb[:], wbt_d[:])
    vt_sb = sb.tile([128, 4 * NR], FP8, tag="vt")
    nc.sync.dma_start(vt_sb[:], vt_d[:])
    up_sb = sb.tile([128, 72 * D_MODEL], FP8, tag="up")
    nc.sync.dma_start(up_sb[:], up_d[:])
    ls_sb = sb.tile([BL, D_MODEL], F32, tag="ls")
    nc.sync.dma_start(ls_sb[:], ls_d[:])
    lb_sb = sb.tile([BL, D_MODEL], F32, tag="lb")
    nc.sync.dma_start(lb_sb[:], lb_d[:])

    if LEVEL <= 0:
        o0 = sb.tile([BL, D_MODEL], F32, tag="o0")
        nc.vector.tensor_scalar_mul(o0[:], zb_sb[:], 2.0)
        nc.sync.dma_start(out_d[:], o0[:])
        return

    identb = sb.tile([128, 128], BF16, tag="identb")
    make_identity(nc, identb[:])

    t_sb = sb.tile([BL, NR], BF16, tag="t")
    q_n = sb.tile([BL, SDK], BF16, tag="q_n")
    qnT = sb.tile([SDK, BL], BF16, tag="qnT")
    h2_sb = sb.tile([BL, D_MODEL], F32, tag="h2")
    knTl = sb.tile([SDK, NL], BF16, tag="knTl")
    knT = sb.tile([SDK, N], BF16, tag="knT")

    with tc.tile_pool(name="ps1", bufs=1, space="PSUM") as ps1:
        # ---------- keys for local 128 pool rows, full D contraction ----
        keys_ps = ps1.tile([NL, SDK], F32, tag="keys")
        for k in range(128):
            nc.tensor.matmul(keys_ps[:],
                             pk_sb[:, k * NL:(k + 1) * NL],
                             wk_sb[:, k * SDK:(k + 1) * SDK],
                             start=(k == 0), stop=(k == 127))
        # normalize along free dim per aspect
        ksq = sb.tile([NL, S], F32, tag="ksq")
        ksc = sb.tile([NL, SDK], F32, tag="sqscr")
        for s in range(S):
            nc.scalar.activation(ksc[:, s * DK:(s + 1) * DK],
                                 keys_ps[:, s * DK:(s + 1) * DK],
                                 AF.Square,
                                 accum_out=ksq[:, s:s + 1])
        knorm = sb.tile([NL, S], F32, tag="knorm")
        nc.scalar.activation(knorm[:], ksq[:], AF.Sqrt)
        nc.vector.tensor_scalar_add(knorm[:], knorm[:], 1e-8)
        krec = sb.tile([NL, S], F32, tag="krec")
        nc.vector.reciprocal(krec[:], knorm[:])
        kn_w = sb.tile([NL, SDK], BF16, tag="kn_w")
        for s in range(S):
            nc.vector.tensor_scalar(
                kn_w[:, s * DK:(s + 1) * DK],
                keys_ps[:, s * DK:(s + 1) * DK],
                krec[:, s:s + 1], None, op0=ALU.mult)
        kn_tp = ps1.tile([SDK, NL], BF16, tag="kn_tp")
        nc.tensor.transpose(kn_tp[:], kn_w[:], identb[:])
        nc.scalar.activation(knTl[:], kn_tp[:], AF.Copy)

        # ---------- AllGather normalized keysT (bf16, 32KB in) ----------
        cc_in = dram.tile([SDK, NL], BF16)
        cc_out = dram.tile([N, NL], BF16)
        nc.sync.dma_start(cc_in[:], knTl[:])
        if NO_CC:
            for c in range(NCORES):
                nc.sync.dma_start(cc_out[c * SDK:(c + 1) * SDK, :], cc_in[:])
        else:
            nc.gpsimd.collective_compute(
                "AllGather", ALU.bypass,
                replica_groups=[list(range(NCORES))],
                ins=[cc_in[:].opt()], outs=[cc_out[:].opt()],
            )
        nc.sync.dma_start(
            knT[:].rearrange("p (c n) -> p c n", c=NCORES),
            cc_out[:].rearrange("(c p) n -> p c n", p=SDK))

        # ---------- queries [b, sdk] + normalize (bf16 path) ----------
        q_ps = ps1.tile([BL, SDK], F32, tag="q")
        for c in range(4):
            nc.tensor.matmul(q_ps[:],
                             zt_sb[:, c * BL:(c + 1) * BL],
                             wq_sb[:, c * SDK:(c + 1) * SDK],
                             start=(c == 0), stop=(c == 3))
        qsq = sb.tile([BL, S], F32, tag="qsq")
        qsc = ksc  # shared scratch, phases are sequential
        for s in range(S):
            nc.scalar.activation(qsc[:, s * DK:(s + 1) * DK],
                                 q_ps[:, s * DK:(s + 1) * DK],
                                 AF.Square,
                                 accum_out=qsq[:, s:s + 1])
        qnorm = sb.tile([BL, S], F32, tag="qnorm")
        nc.scalar.activation(qnorm[:], qsq[:], AF.Sqrt)
        nc.vector.tensor_scalar_add(qnorm[:], qnorm[:], 1e-8)
        qrec = sb.tile([BL, S], F32, tag="qrec")
        nc.vector.reciprocal(qrec[:], qnorm[:])
        # fold softmax(aspect_logits) weights into q_n
        for s, w_s in ((0, w0_f), (1, w1_f)):
            nc.vector.tensor_scalar(
                q_n[:, s * DK:(s + 1) * DK],
                q_ps[:, s * DK:(s + 1) * DK],
                qrec[:, s:s + 1], float(w_s),
                op0=ALU.mult, op1=ALU.mult)

        # ---------- t = z @ V'^T  [b, nr] (independent of CC) ----------
        with tc.tile_pool(name="psT", bufs=4, space="PSUM") as psT:
            for j in range(16):
                t_ps = psT.tile([BL, 512], F32, tag="t")
                for a in range(4):
                    nc.tensor.matmul(
                        t_ps[:],
                        zt_sb[:, a * BL:(a + 1) * BL],
                        vt_sb[:, a * NR + j * 512:a * NR + (j + 1) * 512],
                        start=(a == 0), stop=(a == 3))
                nc.scalar.activation(t_sb[:, j * 512:(j + 1) * 512],
                                     t_ps[:], AF.Copy)

        # ---------- h2 = z @ W_base^T (independent of CC) ----------
        h2_ps = ps1.tile([BL, D_MODEL], F32, tag="h2p")
        for c in range(4):
            nc.tensor.matmul(h2_ps[:],
                             zt_sb[:, c * BL:(c + 1) * BL],
                             wbt_sb[:, c * D_MODEL:(c + 1) * D_MODEL],
                             start=(c == 0), stop=(c == 3))
        nc.scalar.activation(h2_sb[:], h2_ps[:], AF.Copy)

    # ---------- scores (after CC) ----------
    scores = sb.tile([BL, N], F32, tag="scores")
    with tc.tile_pool(name="psS", bufs=2, space="PSUM") as psS:
        q_tp = psS.tile([SDK, BL], BF16, tag="qtp")
        nc.tensor.transpose(q_tp[:], q_n[:], identb[:])
        nc.scalar.activation(qnT[:], q_tp[:], AF.Copy)
        for h in range(2):
            sc_ps = psS.tile([BL, 512], F32, tag="sc")
            nc.tensor.matmul(sc_ps[:], qnT[:],
                             knT[:, h * 512:(h + 1) * 512],
                             start=True, stop=True)
            nc.scalar.activation(scores[:, h * 512:(h + 1) * 512],
                                 sc_ps[:], AF.Copy)

    if LEVEL <= 3:
        nc.sync.dma_start(out_d[:], scores[:, :D_MODEL])
        return

    # ---------- top-16 threshold + alpha (unnormalized) ----------
    m8a = sb.tile([BL, 8], F32, tag="m8a")
    nc.vector.max(out=m8a[:], in_=scores[:])
    s_mr = sb.tile([BL, N], F32, tag="s_mr")
    nc.vector.match_replace(out=s_mr[:], in_to_replace=m8a[:],
                            in_values=scores[:], imm_value=-1e30)
    m8b = sb.tile([BL, 8], F32, tag="m8b")
    nc.vector.max(out=m8b[:], in_=s_mr[:])
    sig_b = sb.tile([BL, 1], F32, tag="sig_b")
    nc.vector.memset(sig_b[:], float(-LAMBDA_SHARP * tau_f))
    sig = sb.tile([BL, N], F32, tag="sig")
    nc.scalar.activation(sig[:], scores[:], AF.Sigmoid,
                         scale=LAMBDA_SHARP, bias=sig_b[:])
    ex = sb.tile([BL, N], F32, tag="ex")
    nc.scalar.activation(ex[:], scores[:], AF.Exp)
    ge = s_mr  # s_mr dead after m8b
    nc.vector.tensor_mul(ge[:], sig[:], ex[:])
    alpha = ex  # ex dead after the ge product
    den = sb.tile([BL, 1], F32, tag="den")
    nc.vector.scalar_tensor_tensor(
        out=alpha[:], in0=scores[:], scalar=m8b[:, 7:8], in1=ge[:],
        op0=ALU.is_ge, op1=ALU.mult, accum_out=den[:])
    # rden = 1 / (SC_H * (den + 1e-8)) folds the fp8 scaling
    den2 = sb.tile([BL, 1], F32, tag="den2")
    nc.vector.tensor_scalar(den2[:], den[:], float(SC_H),
                            float(SC_H * 1e-8), op0=ALU.mult, op1=ALU.add)
    rden = sb.tile([BL, 1], F32, tag="rden")
    nc.vector.reciprocal(rden[:], den2[:])

    if LEVEL <= 4:
        nc.sync.dma_start(out_d[:], alpha[:, :D_MODEL])
        return

    # ---------- s' = [alpha*t, SC_V*alpha] bf16; pipelined h1 ----------
    s_sb = sb.tile([BL, NR + N], BF16, tag="s_sb")
    NG = 72

    def emit_smult(j):
        if j < 16:
            eng = nc.vector if j % 2 == 0 else nc.gpsimd
            eng.tensor_tensor(
                out=s_sb[:, j * 512:(j + 1) * 512]
                    .rearrange("p (n r) -> p n r", r=R),
                in0=t_sb[:, j * 512:(j + 1) * 512]
                    .rearrange("p (n r) -> p n r", r=R),
                in1=alpha[:, j * 64:(j + 1) * 64]
                    .unsqueeze(2).broadcast_to([BL, 64, R]),
                op=ALU.mult)
        else:  # alpha chunk of s'
            h = j - 16
            eng = nc.vector if h % 2 == 0 else nc.gpsimd
            eng.tensor_scalar_mul(
                s_sb[:, NR + h * 512:NR + (h + 1) * 512],
                alpha[:, h * 512:(h + 1) * 512], float(SC_V))

    with tc.tile_pool(name="psH", bufs=1, space="PSUM") as psH, \
         tc.tile_pool(name="psR", bufs=4, space="PSUM") as psR:
        h1_ps = psH.tile([BL, D_MODEL], F32, tag="h1")
        sTs = []

        def emit_transpose(g):
            tr = psR.tile([128, 128], BF16, tag="tr")
            nc.tensor.transpose(tr[:], s_sb[:, g * 128:(g + 1) * 128],
                                identb[:])
            sT = sbr.tile([128, 128], BF16, tag="sT")
            nc.scalar.activation(sT[:], tr[:], AF.Copy)
            sTs.append(sT)

        LAG = 4
        for g in range(NG):
            if g % 4 == 0:
                emit_smult(g // 4)
            emit_transpose(g)
            if g >= LAG:
                nc.tensor.matmul(h1_ps[:], sTs[g - LAG][:],
                                 up_sb[:, (g - LAG) * D_MODEL:
                                       (g - LAG + 1) * D_MODEL],
                                 start=(g == LAG), stop=False)
        for g in range(NG - LAG, NG):
            nc.tensor.matmul(h1_ps[:], sTs[g][:],
                             up_sb[:, g * D_MODEL:(g + 1) * D_MODEL],
                             start=False, stop=(g == NG - 1))

        # ---------- combine + layernorm ----------
        A_sb = sb.tile([BL, D_MODEL], F32, tag="A")
        nc.vector.tensor_scalar(A_sb[:], h1_ps[:], rden[:], None,
                                op0=ALU.mult)
    nc.vector.tensor_add(A_sb[:], A_sb[:], h2_sb[:])
    x_sb = sb.tile([BL, D_MODEL], F32, tag="x")
    nc.vector.scalar_tensor_tensor(
        out=x_sb[:], in0=A_sb[:], scalar=float(gamma_f), in1=zb_sb[:],
        op0=ALU.mult, op1=ALU.add)
    mean = sb.tile([BL, 1], F32, tag="mean")
    nc.vector.reduce_sum(mean[:], x_sb[:], axis=mybir.AxisListType.X)
    nc.vector.tensor_scalar_mul(mean[:], mean[:], 1.0 / D_MODEL)
    xc = sb.tile([BL, D_MODEL], F32, tag="xc")
    nc.vector.tensor_scalar(xc[:], x_sb[:], mean[:], None,
                            op0=ALU.subtract)
    xsq = A_sb  # A dead once x is computed
    ssq = sb.tile([BL, 1], F32, tag="ssq")
    nc.scalar.activation(xsq[:], xc[:], AF.Square, accum_out=ssq[:])
    vare = sb.tile([BL, 1], F32, tag="vare")
    nc.vector.tensor_scalar(vare[:], ssq[:], 1.0 / D_MODEL, LN_EPS,
                            op0=ALU.mult, op1=ALU.add)
    sd = sb.tile([BL, 1], F32, tag="sd")
    nc.scalar.activation(sd[:], vare[:], AF.Sqrt)
    rstd = sb.tile([BL, 1], F32, tag="rstd")
    nc.vector.reciprocal(rstd[:], sd[:])
    y1 = A_sb  # reuse again for the normalized value
    nc.vector.tensor_scalar(y1[:], xc[:], rstd[:], None, op0=ALU.mult)
    out_sb = x_sb  # x dead
    nc.vector.tensor_mul(out_sb[:], y1[:], ls_sb[:])
    nc.vector.tensor_add(out_sb[:], out_sb[:], lb_sb[:])
    nc.sync.dma_start(out_d[:], out_sb[:])


def _pack(x, p=128):
    """[K*p, F] row-chunked -> [p, K*F] (chunk k at cols k*F:(k+1)*F)."""
    k = x.shape[0] // p
    return np.ascontiguousarray(
        x.reshape(k, p, -1).transpose(1, 0, 2).reshape(p, -1))


def kernel(z, pool_vectors, W_Q, W_K, aspect_logits, tau,
           W_base, b_base, gamma, ln_scale, ln_bias):
    global LAST_EXEC_NS
    z = np.asarray(z, np.float32)
    pool = np.asarray(pool_vectors, np.float32)
    W_Q = np.asarray(W_Q, np.float32)
    W_K = np.asarray(W_K, np.float32)
    aspect_logits = np.asarray(aspect_logits, np.float32)
    tau_f = float(np.asarray(tau))
    W_base = np.asarray(W_base, np.float32)
    b_base = np.asarray(b_base, np.float32)
    gamma_f = float(np.asarray(gamma))
    ln_scale = np.asarray(ln_scale, np.float32)
    ln_bias = np.asarray(ln_bias, np.float32)

    e = np.exp(aspect_logits - aspect_logits.max())
    w = e / e.sum()
    w0_f, w1_f = float(w[0]), float(w[1])

    nc = _build(tau_f, w0_f, w1_f, gamma_f)

    fp8 = ml_dtypes.float8_e4m3
    bf16 = ml_dtypes.bfloat16

    # ---- shared host-side layout prep ----
    wk_cat = np.concatenate([W_K[0], W_K[1]], axis=1)          # [D, 128]
    wk = _pack(wk_cat.astype(bf16))                            # [128, 128*128]
    wq = _pack(np.concatenate([W_Q[0], W_Q[1]], axis=1).astype(bf16))
    # V'^T: [a, n*R + r], scaled
    vt = _pack((pool[:, U_END:V_END].reshape(N, R, D_MODEL)
                .transpose(2, 0, 1).reshape(D_MODEL, NR)
                * SC_V).astype(fp8))
    # up' = [SC_U*U_perm; SC_U*bias] [9216, 512], scaled
    up_rows = np.concatenate([
        pool[:, :U_END].reshape(N, D_MODEL, R).transpose(0, 2, 1)
        .reshape(NR, D_MODEL),
        pool[:, V_END:B_END],
    ], axis=0) * SC_U
    up = _pack(up_rows.astype(fp8))                            # [128, 72*512]
    wbt = _pack(np.ascontiguousarray(W_base.T).astype(bf16))   # [128, 4*512]
    ls = np.broadcast_to(ln_scale, (BL, D_MODEL)).astype(np.float32).copy()
    lb = np.broadcast_to(ln_bias, (BL, D_MODEL)).astype(np.float32).copy()
    gb = (gamma_f * b_base).astype(np.float32)

    in_maps = []
    for c in range(NCORES):
        z_loc = np.ascontiguousarray(z[c * BL:(c + 1) * BL])
        zt_loc = _pack(np.ascontiguousarray(z_loc.T).astype(bf16))
        pk_loc = _pack(np.ascontiguousarray(
            pool[c * NL:(c + 1) * NL, :].T).astype(bf16))
        in_maps.append({
            "pk": pk_loc, "wk": wk, "wq": wq, "zt": zt_loc,
            "zb": z_loc + gb, "ls": ls, "lb": lb, "wbt": wbt,
            "vt": vt, "up": up,
        })

    res = run_bass_kernel_spmd(nc, in_maps, core_ids=list(range(NCORES)),
                               trace=TRACE, tmpdir=TMPDIR)
    LAST_EXEC_NS = res.exec_time_ns
    out = np.concatenate([res.results[c]["out"] for c in range(NCORES)],
                         axis=0)
    return out.astype(np.float32)
